# revision 19
# baseline (speedup 1.0000x reference)
"""Bass/Tile kernel for nn_Attention_9234179687166 on 8 TRN2 NeuronCores.

Reference computation per batch b (B=32, L=K=D=1024):
    q      = query @ W_in.T                    # [L, D]
    scores = q @ context.T                     # [L, K]
    w      = masked_softmax(scores, mask)      # multiplicative mask + renorm
    mix    = w @ context                       # [L, D]
    out    = tanh(concat([mix, q]) @ W_out.T)  # [L, D]

Sharding: data-parallel over batch, 4 batches per core, weights replicated.

Per-core program layout (contraction dim always on partitions):
    W_inT[d,e], W_outT[c,d] built once by PE transpose (f16 source - the
    f32->f16 cast happens right behind the weight DMAs, so the setup
    transposes run at the fast f16 weight-load rate).
    Per batch: ctxT[e,k] + ctx_bf[k,d'] (f16). Per l-half: qT[d,l],
    step1 -> qTr[e,l], step2 scores in PSUM, masked softmax (DVE+ACT),
    w = e * (1/sum(e)) pre-normalized on ACT, w transposed to wT[k,l],
    step4 -> mixT[d',l], step5 accumulates BOTH [mixT; qTr] @ W_outT parts
    into one PSUM group and applies tanh straight out of PSUM.

All transposes are 4-packed PE transposes (four 128x128 transposes into one
PSUM tile, one grouped DVE/ACT copy out). DMA-xbar transposes were measured
to anti-overlap with regular DMA traffic (1.6x worse than serial), so
everything stays on the PE where it overlaps the load stream.

Masked softmax (mask m in {0,1}, scores s):
    u = (s + 4096)*m  (masked -> 0); e = exp(u - max(u)) has masked lanes
    exp(-~4096) == 0 exactly, and w = e/sum(e) matches the reference up to
    the +1e-13 term (~1e-10 relative). w is normalized BEFORE the wT
    transpose (ACT copy with per-partition scale), which lets step 5 run as
    a single PSUM accumulation with no separate combine pass.

Software pipeline per half h (PE priority order):
    scores(h) lj=0..3 (wT transposes lag 2 tiles behind the softmax chain)
    -> qT transposes for h+1 -> step1(h+1)  [covers the softmax tail]
    -> step4(h) -> step5(h).
qTr/qT are double-buffered so step1(h+1) can run while step5(h) still
reads qTr(h).
"""

import sys

sys.path.insert(0, "/opt/trn_rl_repo")

import numpy as np

P = 128
D = 1024
TWO_D = 2048
DT = D // P      # 8 tiles over D
CT = TWO_D // P  # 16 tiles over 2D
LARGE = 4096.0
N_CORES = 8
B_FULL = 32
NB = B_FULL // N_CORES  # batches per core

_prog_cache = {}
last_results = None  # BassKernelResults of the most recent kernel() call


def build_program(nb, L, K=1024, reps=1, wlag=2):
    import concourse.mybir as mybir
    import concourse.tile as tile
    from concourse import bacc
    from concourse.masks import make_identity

    f32 = mybir.dt.float32
    f16 = mybir.dt.float16
    i32 = mybir.dt.int32
    Alu = mybir.AluOpType
    Act = mybir.ActivationFunctionType
    KT = K // P
    LH = min(512, L)      # l-half width (free dim of step1/4 matmuls)
    NHALF = L // LH
    LJ = LH // P          # 128-row l tiles per half
    KH = K // 512         # 512-wide k chunks for the scores matmul

    nc = bacc.Bacc("TRN2", target_bir_lowering=False, debug=False,
                   num_devices=N_CORES)
    q_d = nc.dram_tensor("query", [nb, L, D], f32, kind="ExternalInput")
    c_d = nc.dram_tensor("context", [nb, K, D], f32, kind="ExternalInput")
    m_d = nc.dram_tensor("mask", [nb, L, K], i32, kind="ExternalInput")
    win_d = nc.dram_tensor("W_in", [D, D], f32, kind="ExternalInput")
    wout_d = nc.dram_tensor("W_out", [D, TWO_D], f32, kind="ExternalInput")
    out_d = nc.dram_tensor("out", [nb, L, D], f32, kind="ExternalOutput")

    copy_flip = [0]

    def grouped_copy(nc, dst_ap, src_ap):
        # Alternate psum->sbuf copies between DVE and ACT to halve the
        # per-engine copy latency chain behind the PE transposes.
        if copy_flip[0] % 2 == 0:
            nc.vector.tensor_copy(dst_ap, src_ap)
        else:
            nc.scalar.activation(dst_ap, src_ap, mybir.ActivationFunctionType.Copy)
        copy_flip[0] += 1

    with tile.TileContext(nc) as tc:
        with (
            tc.tile_pool(name="const", bufs=1) as constp,
            tc.tile_pool(name="wres", bufs=1) as wres,
            tc.tile_pool(name="ps_big", bufs=2, space="PSUM") as ps_big,
            tc.tile_pool(name="ps_mm", bufs=4, space="PSUM") as ps_mm,
        ):
            ident = constp.tile([P, P], f32)
            make_identity(nc, ident)
            ident_bf = constp.tile([P, P], f16)
            nc.vector.tensor_copy(ident_bf[:], ident[:])

            W_inT = wres.tile([P, DT, D], f16)        # [d_in, d_out, e]
            W_outT = wres.tile([P, CT, D], f16)      # [c_in, c_out, d]

            def transpose_pack4(nc, dst_tile, dst_t0, dst_col0, src_ap_fn, n):
                """n f16 PE transposes (groups of up to 4) of 128x128 slices.
                src_ap_fn(i) gives the i-th source slice; results land in
                dst_tile[:, dst_t0+i, dst_col0:dst_col0+128]."""
                g = 0
                while g < n:
                    gn = min(4, n - g)
                    tp = ps_mm.tile([P, 4 * P], f16, tag="mm")
                    for i in range(gn):
                        nc.tensor.transpose(
                            tp[:, i * P:(i + 1) * P], src_ap_fn(g + i),
                            ident_bf[:])
                    grouped_copy(
                        nc,
                        dst_tile[:, dst_t0 + g:dst_t0 + g + gn,
                                 dst_col0:dst_col0 + P],
                        tp[:, :gn * P],
                    )
                    g += gn

            with (
                tc.tile_pool(name="ctx", bufs=1) as ctxp,
                tc.tile_pool(name="acts", bufs=1) as actsp,
                tc.tile_pool(name="rot", bufs=4) as natp,
                tc.tile_pool(name="sm", bufs=2) as smp,
            ):
                ctx_tiles = {}

                def emit_ctx_stage(b):
                    # context: cast to f16 on the idle GPSIMD, PE-transpose
                    # to ctxT.
                    ctxT = ctxp.tile([P, DT, K], f16, tag="ctxT")      # [e,., k]
                    ctx_bf = ctxp.tile([P, KT, D], f16, tag="ctxbf")  # [k,., d']
                    for ki in range(KT):
                        nat = natp.tile([P, D], f32, tag="nat")
                        nc.sync.dma_start(nat[:], c_d[b, ki * P:(ki + 1) * P, :])
                        nc.gpsimd.tensor_copy(ctx_bf[:, ki, :], nat[:])
                        transpose_pack4(
                            nc, ctxT, 0, ki * P,
                            lambda ei, ki=ki: ctx_bf[:, ki, ei * P:(ei + 1) * P],
                            DT)
                    ctx_tiles[b] = (ctxT, ctx_bf)

                def emit_query_loads(b, h):
                    # Cast each chunk to f16 (GPSIMD) right behind its DMA so
                    # the transposes run single-pass and the nat rotor frees
                    # early.
                    l0 = h * LH
                    qhs = []
                    for lj in range(LJ):
                        nat = natp.tile([P, D], f32, tag="nat")
                        nc.sync.dma_start(
                            nat[:], q_d[b, l0 + lj * P: l0 + (lj + 1) * P, :])
                        qh = smp.tile([P, D], f16, tag="qh", bufs=4)
                        nc.gpsimd.tensor_copy(qh[:], nat[:])
                        qhs.append(qh)
                    return qhs

                def emit_query_transposes(qhs):
                    qT = actsp.tile([P, DT, LH], f16, tag="qT", bufs=2)
                    for lj, qh in enumerate(qhs):
                        transpose_pack4(
                            nc, qT, 0, lj * P,
                            lambda di, qh=qh: qh[:, di * P:(di + 1) * P],
                            DT)
                    return qT

                def emit_step1(qT):
                    # qTr[e, l] = W_inT.T @ qT (f16), e on partitions.
                    qTr = actsp.tile([P, DT, LH], f16, tag="qTr", bufs=2)
                    for ei in range(DT):
                        psq = ps_mm.tile([P, LH], f32, tag="mm")
                        for di in range(DT):
                            nc.tensor.matmul(
                                psq[:],
                                W_inT[:, di, ei * P:(ei + 1) * P],
                                qT[:, di, :],
                                start=(di == 0), stop=(di == DT - 1),
                            )
                        grouped_copy(nc, qTr[:, ei, :], psq[:])
                    return qTr

                def emit_half(b, h, qTr, next_bh, tail_hook=None):
                    l0 = h * LH
                    ctxT, ctx_bf = ctx_tiles[b]

                    # Prefetch the next half's query tiles first so their
                    # DMAs/casts finish before the tail needs the transposes.
                    next_qhs = (emit_query_loads(*next_bh)
                                if next_bh is not None else None)

                    # ---- step 2 + masked softmax; wT transposes lag wlag
                    # l-tiles so the softmax chain hides under the next
                    # tiles' score matmuls. w is pre-normalized by
                    # rec = 1/sum(e) on ACT before the transpose. ----
                    wT = actsp.tile([P, KT, LH], f16, tag="wT")
                    w_tiles = [None] * LJ

                    def emit_w_transpose(lj):
                        w_sb = w_tiles[lj]
                        for g in range(KT // 4):
                            tpb = ps_mm.tile([P, 4 * P], f16, tag="mm")
                            for i in range(4):
                                ki = g * 4 + i
                                nc.tensor.transpose(
                                    tpb[:, i * P:(i + 1) * P],
                                    w_sb[:, ki * P:(ki + 1) * P], ident_bf[:])
                            grouped_copy(
                                nc,
                                wT[:, g * 4:(g + 1) * 4, lj * P:(lj + 1) * P],
                                tpb[:])

                    for lj in range(LJ):
                        mi = smp.tile([P, K], i32, tag="mask", bufs=2)
                        nc.sync.dma_start(
                            mi[:], m_d[b, l0 + lj * P: l0 + (lj + 1) * P, :])
                        pss = ps_big.tile([P, K], f32, tag="big")
                        for ei in range(DT):
                            for kh in range(KH):
                                nc.tensor.matmul(
                                    pss[:, kh * 512:(kh + 1) * 512],
                                    qTr[:, ei, lj * P:(lj + 1) * P],
                                    ctxT[:, ei, kh * 512:(kh + 1) * 512],
                                    start=(ei == 0), stop=(ei == DT - 1),
                                )
                        st = smp.tile([P, 4], f32, tag="stats", bufs=2)
                        # u = (s + LARGE) * m in SBUF frees the scores PSUM
                        # right after this op.
                        u_t = smp.tile([P, K], f32, tag="u", bufs=2)
                        nc.vector.scalar_tensor_tensor(
                            u_t[:], pss[:], LARGE, mi[:],
                            op0=Alu.add, op1=Alu.mult)
                        nc.vector.tensor_reduce(
                            st[:, 0:1], u_t[:], axis=mybir.AxisListType.X,
                            op=Alu.max, negate=True)
                        e_sb = smp.tile([P, K], f16, tag="e", bufs=2)
                        nc.scalar.activation(
                            e_sb[:], u_t[:], Act.Exp,
                            bias=st[:, 0:1], accum_out=st[:, 1:2])
                        nc.vector.reciprocal(st[:, 2:3], st[:, 1:2])
                        # w = e * rec on the idle GPSIMD (keeps DVE/ACT free
                        # for the reduce/exp/copy traffic).
                        w_sb = smp.tile([P, K], f16, tag="w", bufs=3)
                        nc.gpsimd.tensor_scalar(
                            w_sb[:], e_sb[:], st[:, 2:3], None, op0=Alu.mult)
                        w_tiles[lj] = w_sb
                        if lj >= wlag:
                            emit_w_transpose(lj - wlag)

                    # Lagged tail: pending wT transposes, the next half's
                    # query transposes, then step1(h+1) keep the PE fed
                    # while the last softmax chains drain.
                    if tail_hook is not None:
                        tail_hook()
                    if wlag == 2:
                        emit_w_transpose(LJ - 2)
                    qT_next = (emit_query_transposes(next_qhs)
                               if next_qhs is not None else None)
                    emit_w_transpose(LJ - 1)
                    qTr_next = emit_step1(qT_next) if qT_next is not None else None

                    # ---- step 4: mixT[d', l] = ctx_bf.T @ wT (f16) ----
                    mixT = actsp.tile([P, DT, LH], f16, tag="mixT")
                    for di in range(DT):
                        psm = ps_mm.tile([P, LH], f32, tag="mm")
                        for ki in range(KT):
                            nc.tensor.matmul(
                                psm[:],
                                ctx_bf[:, ki, di * P:(di + 1) * P],
                                wT[:, ki, :],
                                start=(ki == 0), stop=(ki == KT - 1),
                            )
                        grouped_copy(nc, mixT[:, di, :], psm[:])

                    # ---- step 5: out = tanh([mixT; qTr] @ W_outT), one PSUM
                    # accumulation per lj (w pre-normalized). ----
                    for lj in range(LJ):
                        pso = ps_big.tile([P, K], f32, tag="big")
                        for ci in range(DT):
                            lhs = mixT[:, ci, lj * P:(lj + 1) * P]
                            for dh in range(D // 512):
                                nc.tensor.matmul(
                                    pso[:, dh * 512:(dh + 1) * 512], lhs,
                                    W_outT[:, ci, dh * 512:(dh + 1) * 512],
                                    start=(ci == 0), stop=False,
                                )
                        for ci in range(DT):
                            lhs = qTr[:, ci, lj * P:(lj + 1) * P]
                            for dh in range(D // 512):
                                nc.tensor.matmul(
                                    pso[:, dh * 512:(dh + 1) * 512], lhs,
                                    W_outT[:, DT + ci,
                                           dh * 512:(dh + 1) * 512],
                                    start=False, stop=(ci == DT - 1),
                                )
                        for dh in range(D // 512):
                            o_sb = smp.tile([P, 512], f32, tag="osb", bufs=2)
                            nc.scalar.activation(
                                o_sb[:], pso[:, dh * 512:(dh + 1) * 512],
                                Act.Tanh)
                            nc.sync.dma_start(
                                out_d[b, l0 + lj * P: l0 + (lj + 1) * P,
                                      dh * 512:(dh + 1) * 512],
                                o_sb[:])
                    return qTr_next

                def emit_w_in_setup():
                    for ei in range(DT):
                        nat = natp.tile([P, D], f32, tag="nat")
                        nc.sync.dma_start(nat[:], win_d[ei * P:(ei + 1) * P, :])
                        nat16 = natp.tile([P, D], f16, tag="nat16", bufs=2)
                        nc.vector.tensor_copy(nat16[:], nat[:])
                        transpose_pack4(
                            nc, W_inT, 0, ei * P,
                            lambda di, t=nat16: t[:, di * P:(di + 1) * P],
                            DT)

                def emit_w_out_setup():
                    for di in range(DT):
                        for half in range(2):
                            nat = natp.tile([P, D], f32, tag="nat")
                            nc.sync.dma_start(
                                nat[:],
                                wout_d[di * P:(di + 1) * P,
                                       half * D:(half + 1) * D])
                            nat16 = natp.tile([P, D], f16, tag="nat16", bufs=2)
                            nc.vector.tensor_copy(nat16[:], nat[:])
                            transpose_pack4(
                                nc, W_outT, 8 * half, di * P,
                                lambda ci, t=nat16: t[:, ci * P:(ci + 1) * P],
                                DT)

                def emit_all():
                    emit_w_in_setup()
                    emit_ctx_stage(0)
                    qT = emit_query_transposes(emit_query_loads(0, 0))
                    qTr = emit_step1(qT)
                    halves = [(b, h) for b in range(nb) for h in range(NHALF)]
                    for i, (b, h) in enumerate(halves):
                        if h == 0 and b > 0:
                            emit_ctx_stage(b)
                            ctx_tiles.pop(b - 1)
                        nxt = halves[i + 1] if i + 1 < len(halves) else None
                        # W_out is only needed by step5 of the first half;
                        # emitting it in that half's tail keeps its 8 MB of
                        # DMA behind the first masks/queries in the queue.
                        hook = emit_w_out_setup if i == 0 else None
                        qTr = emit_half(b, h, qTr, nxt, tail_hook=hook)

                if reps == 1:
                    emit_all()
                else:
                    with tc.For_i(0, reps, 1):
                        emit_all()

    nc.compile()
    return nc


def _get_program(nb, L):
    key = (nb, L)
    if key not in _prog_cache:
        _prog_cache[key] = build_program(nb, L)
    return _prog_cache[key]


def kernel(query, context, mask, W_in, W_out):
    from concourse.bass_utils import run_bass_kernel_spmd

    query = np.ascontiguousarray(query, dtype=np.float32)
    context = np.ascontiguousarray(context, dtype=np.float32)
    W_in = np.ascontiguousarray(W_in, dtype=np.float32)
    W_out = np.ascontiguousarray(W_out, dtype=np.float32)
    B, L, _ = query.shape
    mask3 = np.ascontiguousarray(mask.reshape(B, L, -1), dtype=np.int32)

    nb = B // N_CORES
    nc = _get_program(nb, L)
    in_maps = []
    for c in range(N_CORES):
        b0 = c * nb
        in_maps.append({
            "query": query[b0:b0 + nb],
            "context": context[b0:b0 + nb],
            "mask": mask3[b0:b0 + nb],
            "W_in": W_in,
            "W_out": W_out,
        })
    res = run_bass_kernel_spmd(nc, in_maps, core_ids=list(range(N_CORES)))
    global last_results
    last_results = res
    out = np.concatenate([r["out"] for r in res.results], axis=0)
    return out


# revision 20
# speedup vs baseline: 1.6586x; 1.6586x over previous
"""Bass/Tile kernel for nn_Attention_9234179687166 on 8 TRN2 NeuronCores.

Reference computation per batch b (B=32, L=K=D=1024):
    q      = query @ W_in.T                    # [L, D]
    scores = q @ context.T                     # [L, K]
    w      = masked_softmax(scores, mask)      # multiplicative mask + renorm
    mix    = w @ context                       # [L, D]
    out    = tanh(concat([mix, q]) @ W_out.T)  # [L, D]

Sharding: data-parallel over batch, 4 batches per core, weights replicated.

Per-core program layout (contraction dim always on partitions):
    W_inT[d,e], W_outT[c,d] built once by PE transpose (f16 source - the
    f32->f16 cast happens right behind the weight DMAs, so the setup
    transposes run at the fast f16 weight-load rate).
    Per batch: ctxT[e,k] + ctx_bf[k,d'] (f16). Per l-half: qT[d,l],
    step1 -> qTr[e,l], step2 scores in PSUM, masked softmax (DVE+ACT),
    w = e * (1/sum(e)) pre-normalized on ACT, w transposed to wT[k,l],
    step4 -> mixT[d',l], step5 accumulates BOTH [mixT; qTr] @ W_outT parts
    into one PSUM group and applies tanh straight out of PSUM.

All transposes are 4-packed PE transposes (four 128x128 transposes into one
PSUM tile, one grouped DVE/ACT copy out). DMA-xbar transposes were measured
to anti-overlap with regular DMA traffic (1.6x worse than serial), so
everything stays on the PE where it overlaps the load stream.

Masked softmax (mask m in {0,1}, scores s):
    u = (s + 4096)*m  (masked -> 0); e = exp(u - max(u)) has masked lanes
    exp(-~4096) == 0 exactly, and w = e/sum(e) matches the reference up to
    the +1e-13 term (~1e-10 relative). w is normalized BEFORE the wT
    transpose (ACT copy with per-partition scale), which lets step 5 run as
    a single PSUM accumulation with no separate combine pass.

Software pipeline per half h (PE priority order):
    scores(h) lj=0..3 (wT transposes lag 2 tiles behind the softmax chain)
    -> qT transposes for h+1 -> step1(h+1)  [covers the softmax tail]
    -> step4(h) -> step5(h).
qTr/qT are double-buffered so step1(h+1) can run while step5(h) still
reads qTr(h).
"""

import sys

sys.path.insert(0, "/opt/trn_rl_repo")

import numpy as np

P = 128
D = 1024
TWO_D = 2048
DT = D // P      # 8 tiles over D
CT = TWO_D // P  # 16 tiles over 2D
LARGE = 4096.0
N_CORES = 8
B_FULL = 32
NB = B_FULL // N_CORES  # batches per core

_prog_cache = {}
last_results = None  # BassKernelResults of the most recent kernel() call


def build_program(nb, L, K=1024, reps=1, wlag=2):
    import concourse.mybir as mybir
    import concourse.tile as tile
    from concourse import bacc
    from concourse.masks import make_identity

    f32 = mybir.dt.float32
    f16 = mybir.dt.float16
    i32 = mybir.dt.int32
    Alu = mybir.AluOpType
    Act = mybir.ActivationFunctionType
    KT = K // P
    LH = min(512, L)      # l-half width (free dim of step1/4 matmuls)
    NHALF = L // LH
    LJ = LH // P          # 128-row l tiles per half
    KH = K // 512         # 512-wide k chunks for the scores matmul

    nc = bacc.Bacc("TRN2", target_bir_lowering=False, debug=False,
                   num_devices=N_CORES)
    q_d = nc.dram_tensor("query", [nb, L, D], f32, kind="ExternalInput")
    c_d = nc.dram_tensor("context", [nb, K, D], f32, kind="ExternalInput")
    m_d = nc.dram_tensor("mask", [nb, L, K], i32, kind="ExternalInput")
    win_d = nc.dram_tensor("W_in", [D, D], f32, kind="ExternalInput")
    wout_d = nc.dram_tensor("W_out", [D, TWO_D], f32, kind="ExternalInput")
    out_d = nc.dram_tensor("out", [nb, L, D], f32, kind="ExternalOutput")

    copy_flip = [0]

    def grouped_copy(nc, dst_ap, src_ap):
        # Alternate psum->sbuf copies between DVE and ACT to halve the
        # per-engine copy latency chain behind the PE transposes.
        if copy_flip[0] % 2 == 0:
            nc.vector.tensor_copy(dst_ap, src_ap)
        else:
            nc.scalar.activation(dst_ap, src_ap, mybir.ActivationFunctionType.Copy)
        copy_flip[0] += 1

    with tile.TileContext(nc) as tc:
        with (
            tc.tile_pool(name="const", bufs=1) as constp,
            tc.tile_pool(name="wres", bufs=1) as wres,
            tc.tile_pool(name="ps_big", bufs=2, space="PSUM") as ps_big,
            tc.tile_pool(name="ps_mm", bufs=4, space="PSUM") as ps_mm,
        ):
            ident = constp.tile([P, P], f32)
            make_identity(nc, ident)
            ident_bf = constp.tile([P, P], f16)
            nc.vector.tensor_copy(ident_bf[:], ident[:])

            W_inT = wres.tile([P, DT, D], f16)        # [d_in, d_out, e]
            W_outT = wres.tile([P, CT, D], f16)      # [c_in, c_out, d]

            def transpose_pack4(nc, dst_tile, dst_t0, dst_col0, src_ap_fn, n):
                """n f16 PE transposes (groups of up to 4) of 128x128 slices.
                src_ap_fn(i) gives the i-th source slice; results land in
                dst_tile[:, dst_t0+i, dst_col0:dst_col0+128]."""
                g = 0
                while g < n:
                    gn = min(4, n - g)
                    tp = ps_mm.tile([P, 4 * P], f16, tag="mm")
                    for i in range(gn):
                        nc.tensor.transpose(
                            tp[:, i * P:(i + 1) * P], src_ap_fn(g + i),
                            ident_bf[:])
                    grouped_copy(
                        nc,
                        dst_tile[:, dst_t0 + g:dst_t0 + g + gn,
                                 dst_col0:dst_col0 + P],
                        tp[:, :gn * P],
                    )
                    g += gn

            with (
                tc.tile_pool(name="ctx", bufs=1) as ctxp,
                tc.tile_pool(name="acts", bufs=1) as actsp,
                tc.tile_pool(name="rot", bufs=4) as natp,
                tc.tile_pool(name="sm", bufs=2) as smp,
            ):
                ctx_tiles = {}

                def emit_ctx_stage(b):
                    # context: cast to f16 (ACT), PE-transpose to ctxT.
                    ctxT = ctxp.tile([P, DT, K], f16, tag="ctxT")      # [e,., k]
                    ctx_bf = ctxp.tile([P, KT, D], f16, tag="ctxbf")  # [k,., d']
                    for ki in range(KT):
                        nat = natp.tile([P, D], f32, tag="nat")
                        nc.sync.dma_start(nat[:], c_d[b, ki * P:(ki + 1) * P, :])
                        nc.scalar.activation(ctx_bf[:, ki, :], nat[:], Act.Copy)
                        transpose_pack4(
                            nc, ctxT, 0, ki * P,
                            lambda ei, ki=ki: ctx_bf[:, ki, ei * P:(ei + 1) * P],
                            DT)
                    ctx_tiles[b] = (ctxT, ctx_bf)

                def emit_query_loads(b, h):
                    # Cast each chunk to f16 (DVE) right behind its DMA so
                    # the transposes run single-pass and the nat rotor frees
                    # early.
                    l0 = h * LH
                    qhs = []
                    for lj in range(LJ):
                        nat = natp.tile([P, D], f32, tag="nat")
                        nc.sync.dma_start(
                            nat[:], q_d[b, l0 + lj * P: l0 + (lj + 1) * P, :])
                        qh = smp.tile([P, D], f16, tag="qh", bufs=4)
                        nc.vector.tensor_copy(qh[:], nat[:])
                        qhs.append(qh)
                    return qhs

                def emit_query_transposes(qhs):
                    qT = actsp.tile([P, DT, LH], f16, tag="qT", bufs=2)
                    for lj, qh in enumerate(qhs):
                        transpose_pack4(
                            nc, qT, 0, lj * P,
                            lambda di, qh=qh: qh[:, di * P:(di + 1) * P],
                            DT)
                    return qT

                def emit_step1(qT):
                    # qTr[e, l] = W_inT.T @ qT (f16), e on partitions.
                    qTr = actsp.tile([P, DT, LH], f16, tag="qTr", bufs=2)
                    for ei in range(DT):
                        psq = ps_mm.tile([P, LH], f32, tag="mm")
                        for di in range(DT):
                            nc.tensor.matmul(
                                psq[:],
                                W_inT[:, di, ei * P:(ei + 1) * P],
                                qT[:, di, :],
                                start=(di == 0), stop=(di == DT - 1),
                            )
                        grouped_copy(nc, qTr[:, ei, :], psq[:])
                    return qTr

                def emit_half(b, h, qTr, next_bh, tail_hook=None):
                    l0 = h * LH
                    ctxT, ctx_bf = ctx_tiles[b]

                    # Prefetch the next half's query tiles first so their
                    # DMAs/casts finish before the tail needs the transposes.
                    next_qhs = (emit_query_loads(*next_bh)
                                if next_bh is not None else None)

                    # ---- step 2 + masked softmax; wT transposes lag wlag
                    # l-tiles so the softmax chain hides under the next
                    # tiles' score matmuls. w is pre-normalized by
                    # rec = 1/sum(e) on ACT before the transpose. ----
                    wT = actsp.tile([P, KT, LH], f16, tag="wT")
                    w_tiles = [None] * LJ

                    def emit_w_transpose(lj):
                        w_sb = w_tiles[lj]
                        for g in range(KT // 4):
                            tpb = ps_mm.tile([P, 4 * P], f16, tag="mm")
                            for i in range(4):
                                ki = g * 4 + i
                                nc.tensor.transpose(
                                    tpb[:, i * P:(i + 1) * P],
                                    w_sb[:, ki * P:(ki + 1) * P], ident_bf[:])
                            grouped_copy(
                                nc,
                                wT[:, g * 4:(g + 1) * 4, lj * P:(lj + 1) * P],
                                tpb[:])

                    for lj in range(LJ):
                        mi = smp.tile([P, K], i32, tag="mask", bufs=2)
                        nc.sync.dma_start(
                            mi[:], m_d[b, l0 + lj * P: l0 + (lj + 1) * P, :])
                        pss = ps_big.tile([P, K], f32, tag="big")
                        for ei in range(DT):
                            for kh in range(KH):
                                nc.tensor.matmul(
                                    pss[:, kh * 512:(kh + 1) * 512],
                                    qTr[:, ei, lj * P:(lj + 1) * P],
                                    ctxT[:, ei, kh * 512:(kh + 1) * 512],
                                    start=(ei == 0), stop=(ei == DT - 1),
                                )
                        st = smp.tile([P, 4], f32, tag="stats", bufs=2)
                        # u = (s + LARGE) * m in SBUF frees the scores PSUM
                        # right after this op.
                        u_t = smp.tile([P, K], f32, tag="u", bufs=2)
                        nc.vector.scalar_tensor_tensor(
                            u_t[:], pss[:], LARGE, mi[:],
                            op0=Alu.add, op1=Alu.mult)
                        nc.vector.tensor_reduce(
                            st[:, 0:1], u_t[:], axis=mybir.AxisListType.X,
                            op=Alu.max, negate=True)
                        e_sb = smp.tile([P, K], f16, tag="e", bufs=2)
                        nc.scalar.activation(
                            e_sb[:], u_t[:], Act.Exp,
                            bias=st[:, 0:1], accum_out=st[:, 1:2])
                        nc.vector.reciprocal(st[:, 2:3], st[:, 1:2])
                        # w = e * rec (pre-normalized so step 5 is a single
                        # PSUM accumulation).
                        w_sb = smp.tile([P, K], f16, tag="w", bufs=3)
                        nc.scalar.activation(
                            w_sb[:], e_sb[:], Act.Copy, scale=st[:, 2:3])
                        w_tiles[lj] = w_sb
                        if lj >= wlag:
                            emit_w_transpose(lj - wlag)

                    # Lagged tail: pending wT transposes, the next half's
                    # query transposes, then step1(h+1) keep the PE fed
                    # while the last softmax chains drain.
                    if tail_hook is not None:
                        tail_hook()
                    if wlag == 2:
                        emit_w_transpose(LJ - 2)
                    qT_next = (emit_query_transposes(next_qhs)
                               if next_qhs is not None else None)
                    emit_w_transpose(LJ - 1)
                    qTr_next = emit_step1(qT_next) if qT_next is not None else None

                    # ---- step 4: mixT[d', l] = ctx_bf.T @ wT (f16) ----
                    mixT = actsp.tile([P, DT, LH], f16, tag="mixT")
                    for di in range(DT):
                        psm = ps_mm.tile([P, LH], f32, tag="mm")
                        for ki in range(KT):
                            nc.tensor.matmul(
                                psm[:],
                                ctx_bf[:, ki, di * P:(di + 1) * P],
                                wT[:, ki, :],
                                start=(ki == 0), stop=(ki == KT - 1),
                            )
                        grouped_copy(nc, mixT[:, di, :], psm[:])

                    # ---- step 5: out = tanh([mixT; qTr] @ W_outT), one PSUM
                    # accumulation per lj (w pre-normalized). ----
                    for lj in range(LJ):
                        pso = ps_big.tile([P, K], f32, tag="big")
                        for ci in range(DT):
                            lhs = mixT[:, ci, lj * P:(lj + 1) * P]
                            for dh in range(D // 512):
                                nc.tensor.matmul(
                                    pso[:, dh * 512:(dh + 1) * 512], lhs,
                                    W_outT[:, ci, dh * 512:(dh + 1) * 512],
                                    start=(ci == 0), stop=False,
                                )
                        for ci in range(DT):
                            lhs = qTr[:, ci, lj * P:(lj + 1) * P]
                            for dh in range(D // 512):
                                nc.tensor.matmul(
                                    pso[:, dh * 512:(dh + 1) * 512], lhs,
                                    W_outT[:, DT + ci,
                                           dh * 512:(dh + 1) * 512],
                                    start=False, stop=(ci == DT - 1),
                                )
                        for dh in range(D // 512):
                            o_sb = smp.tile([P, 512], f32, tag="osb", bufs=2)
                            nc.scalar.activation(
                                o_sb[:], pso[:, dh * 512:(dh + 1) * 512],
                                Act.Tanh)
                            nc.sync.dma_start(
                                out_d[b, l0 + lj * P: l0 + (lj + 1) * P,
                                      dh * 512:(dh + 1) * 512],
                                o_sb[:])
                    return qTr_next

                def emit_w_in_setup():
                    for ei in range(DT):
                        nat = natp.tile([P, D], f32, tag="nat")
                        nc.sync.dma_start(nat[:], win_d[ei * P:(ei + 1) * P, :])
                        nat16 = natp.tile([P, D], f16, tag="nat16", bufs=2)
                        nc.vector.tensor_copy(nat16[:], nat[:])
                        transpose_pack4(
                            nc, W_inT, 0, ei * P,
                            lambda di, t=nat16: t[:, di * P:(di + 1) * P],
                            DT)

                def emit_w_out_setup():
                    for di in range(DT):
                        for half in range(2):
                            nat = natp.tile([P, D], f32, tag="nat")
                            nc.sync.dma_start(
                                nat[:],
                                wout_d[di * P:(di + 1) * P,
                                       half * D:(half + 1) * D])
                            nat16 = natp.tile([P, D], f16, tag="nat16", bufs=2)
                            nc.vector.tensor_copy(nat16[:], nat[:])
                            transpose_pack4(
                                nc, W_outT, 8 * half, di * P,
                                lambda ci, t=nat16: t[:, ci * P:(ci + 1) * P],
                                DT)

                def emit_all():
                    emit_w_in_setup()
                    emit_ctx_stage(0)
                    qT = emit_query_transposes(emit_query_loads(0, 0))
                    qTr = emit_step1(qT)
                    halves = [(b, h) for b in range(nb) for h in range(NHALF)]
                    for i, (b, h) in enumerate(halves):
                        if h == 0 and b > 0:
                            emit_ctx_stage(b)
                            ctx_tiles.pop(b - 1)
                        nxt = halves[i + 1] if i + 1 < len(halves) else None
                        # W_out is only needed by step5 of the first half;
                        # emitting it in that half's tail keeps its 8 MB of
                        # DMA behind the first masks/queries in the queue.
                        hook = emit_w_out_setup if i == 0 else None
                        qTr = emit_half(b, h, qTr, nxt, tail_hook=hook)

                if reps == 1:
                    emit_all()
                else:
                    with tc.For_i(0, reps, 1):
                        emit_all()

    nc.compile()
    return nc


def _get_program(nb, L):
    key = (nb, L)
    if key not in _prog_cache:
        _prog_cache[key] = build_program(nb, L)
    return _prog_cache[key]


def kernel(query, context, mask, W_in, W_out):
    from concourse.bass_utils import run_bass_kernel_spmd

    query = np.ascontiguousarray(query, dtype=np.float32)
    context = np.ascontiguousarray(context, dtype=np.float32)
    W_in = np.ascontiguousarray(W_in, dtype=np.float32)
    W_out = np.ascontiguousarray(W_out, dtype=np.float32)
    B, L, _ = query.shape
    mask3 = np.ascontiguousarray(mask.reshape(B, L, -1), dtype=np.int32)

    nb = B // N_CORES
    nc = _get_program(nb, L)
    in_maps = []
    for c in range(N_CORES):
        b0 = c * nb
        in_maps.append({
            "query": query[b0:b0 + nb],
            "context": context[b0:b0 + nb],
            "mask": mask3[b0:b0 + nb],
            "W_in": W_in,
            "W_out": W_out,
        })
    res = run_bass_kernel_spmd(nc, in_maps, core_ids=list(range(N_CORES)))
    global last_results
    last_results = res
    out = np.concatenate([r["out"] for r in res.results], axis=0)
    return out


# revision 22
# speedup vs baseline: 1.6877x; 1.0175x over previous
"""Bass/Tile kernel for nn_Attention_9234179687166 on 8 TRN2 NeuronCores.

Reference computation per batch b (B=32, L=K=D=1024):
    q      = query @ W_in.T                    # [L, D]
    scores = q @ context.T                     # [L, K]
    w      = masked_softmax(scores, mask)      # multiplicative mask + renorm
    mix    = w @ context                       # [L, D]
    out    = tanh(concat([mix, q]) @ W_out.T)  # [L, D]

Sharding: data-parallel over batch, 4 batches per core, weights replicated.

Per-core program layout (contraction dim always on partitions):
    W_inT[d,e], W_outT[c,d] built once by PE transpose (f16 source - the
    f32->f16 cast happens right behind the weight DMAs, so the setup
    transposes run at the fast f16 weight-load rate).
    Per batch: ctxT[e,k] + ctx_bf[k,d'] (f16). Per l-half: qT[d,l],
    step1 -> qTr[e,l], step2 scores in PSUM, masked softmax (DVE+ACT),
    w = e * (1/sum(e)) pre-normalized on ACT, w transposed to wT[k,l],
    step4 -> mixT[d',l], step5 accumulates BOTH [mixT; qTr] @ W_outT parts
    into one PSUM group and applies tanh straight out of PSUM.

All transposes are 4-packed PE transposes (four 128x128 transposes into one
PSUM tile, one grouped DVE/ACT copy out). DMA-xbar transposes were measured
to anti-overlap with regular DMA traffic (1.6x worse than serial), so
everything stays on the PE where it overlaps the load stream.

Masked softmax (mask m in {0,1}, scores s):
    u = (s + 4096)*m  (masked -> 0); e = exp(u - max(u)) has masked lanes
    exp(-~4096) == 0 exactly, and w = e/sum(e) matches the reference up to
    the +1e-13 term (~1e-10 relative). w is normalized BEFORE the wT
    transpose (ACT copy with per-partition scale), which lets step 5 run as
    a single PSUM accumulation with no separate combine pass.

Software pipeline per half h (PE priority order):
    scores(h) lj=0..3 (wT transposes lag 2 tiles behind the softmax chain)
    -> qT transposes for h+1 -> step1(h+1)  [covers the softmax tail]
    -> step4(h) -> step5(h).
qTr/qT are double-buffered so step1(h+1) can run while step5(h) still
reads qTr(h).
"""

import sys

sys.path.insert(0, "/opt/trn_rl_repo")

import numpy as np

P = 128
D = 1024
TWO_D = 2048
DT = D // P      # 8 tiles over D
CT = TWO_D // P  # 16 tiles over 2D
LARGE = 4096.0
N_CORES = 8
B_FULL = 32
NB = B_FULL // N_CORES  # batches per core

_prog_cache = {}
last_results = None  # BassKernelResults of the most recent kernel() call


def build_program(nb, L, K=1024, reps=1, wlag=2):
    import concourse.mybir as mybir
    import concourse.tile as tile
    from concourse import bacc
    from concourse.masks import make_identity

    f32 = mybir.dt.float32
    f16 = mybir.dt.float16
    i32 = mybir.dt.int32
    Alu = mybir.AluOpType
    Act = mybir.ActivationFunctionType
    KT = K // P
    LH = min(512, L)      # l-half width (free dim of step1/4 matmuls)
    NHALF = L // LH
    LJ = LH // P          # 128-row l tiles per half
    KH = K // 512         # 512-wide k chunks for the scores matmul

    nc = bacc.Bacc("TRN2", target_bir_lowering=False, debug=False,
                   num_devices=N_CORES)
    q_d = nc.dram_tensor("query", [nb, L, D], f32, kind="ExternalInput")
    c_d = nc.dram_tensor("context", [nb, K, D], f32, kind="ExternalInput")
    m_d = nc.dram_tensor("mask", [nb, L, K], i32, kind="ExternalInput")
    win_d = nc.dram_tensor("W_in", [D, D], f32, kind="ExternalInput")
    wout_d = nc.dram_tensor("W_out", [D, TWO_D], f32, kind="ExternalInput")
    out_d = nc.dram_tensor("out", [nb, L, D], f32, kind="ExternalOutput")

    copy_flip = [0]

    def grouped_copy(nc, dst_ap, src_ap):
        # Alternate psum->sbuf copies between DVE and ACT to halve the
        # per-engine copy latency chain behind the PE transposes.
        if copy_flip[0] % 2 == 0:
            nc.vector.tensor_copy(dst_ap, src_ap)
        else:
            nc.scalar.activation(dst_ap, src_ap, mybir.ActivationFunctionType.Copy)
        copy_flip[0] += 1

    with tile.TileContext(nc) as tc:
        with (
            tc.tile_pool(name="const", bufs=1) as constp,
            tc.tile_pool(name="wres", bufs=1) as wres,
            tc.tile_pool(name="ps_big", bufs=2, space="PSUM") as ps_big,
            tc.tile_pool(name="ps_mm", bufs=4, space="PSUM") as ps_mm,
        ):
            ident = constp.tile([P, P], f32)
            make_identity(nc, ident)
            ident_bf = constp.tile([P, P], f16)
            nc.vector.tensor_copy(ident_bf[:], ident[:])

            W_inT = wres.tile([P, DT, D], f16)        # [d_in, d_out, e]
            W_outT = wres.tile([P, CT, D], f16)      # [c_in, c_out, d]

            def transpose_pack4(nc, dst_tile, dst_t0, dst_col0, src_ap_fn, n):
                """n f16 PE transposes (groups of up to 4) of 128x128 slices.
                src_ap_fn(i) gives the i-th source slice; results land in
                dst_tile[:, dst_t0+i, dst_col0:dst_col0+128]."""
                g = 0
                while g < n:
                    gn = min(4, n - g)
                    tp = ps_mm.tile([P, 4 * P], f16, tag="mm")
                    for i in range(gn):
                        nc.tensor.transpose(
                            tp[:, i * P:(i + 1) * P], src_ap_fn(g + i),
                            ident_bf[:])
                    grouped_copy(
                        nc,
                        dst_tile[:, dst_t0 + g:dst_t0 + g + gn,
                                 dst_col0:dst_col0 + P],
                        tp[:, :gn * P],
                    )
                    g += gn

            with (
                tc.tile_pool(name="ctx", bufs=1) as ctxp,
                tc.tile_pool(name="acts", bufs=1) as actsp,
                tc.tile_pool(name="rot", bufs=4) as natp,
                tc.tile_pool(name="sm", bufs=2) as smp,
            ):
                ctx_tiles = {}

                def emit_ctx_stage(b):
                    # context: cast to f16 (ACT), PE-transpose to ctxT.
                    ctxT = ctxp.tile([P, DT, K], f16, tag="ctxT")      # [e,., k]
                    ctx_bf = ctxp.tile([P, KT, D], f16, tag="ctxbf")  # [k,., d']
                    for ki in range(KT):
                        nat = natp.tile([P, D], f32, tag="nat")
                        nc.sync.dma_start(nat[:], c_d[b, ki * P:(ki + 1) * P, :])
                        nc.scalar.activation(ctx_bf[:, ki, :], nat[:], Act.Copy)
                        transpose_pack4(
                            nc, ctxT, 0, ki * P,
                            lambda ei, ki=ki: ctx_bf[:, ki, ei * P:(ei + 1) * P],
                            DT)
                    ctx_tiles[b] = (ctxT, ctx_bf)

                def emit_query_loads(b, h):
                    # Cast each chunk to f16 (DVE) right behind its DMA so
                    # the transposes run single-pass and the nat rotor frees
                    # early.
                    l0 = h * LH
                    qhs = []
                    for lj in range(LJ):
                        nat = natp.tile([P, D], f32, tag="nat")
                        nc.sync.dma_start(
                            nat[:], q_d[b, l0 + lj * P: l0 + (lj + 1) * P, :])
                        qh = smp.tile([P, D], f16, tag="qh", bufs=4)
                        nc.vector.tensor_copy(qh[:], nat[:])
                        qhs.append(qh)
                    return qhs

                def emit_query_transposes(qhs):
                    qT = actsp.tile([P, DT, LH], f16, tag="qT", bufs=2)
                    for lj, qh in enumerate(qhs):
                        transpose_pack4(
                            nc, qT, 0, lj * P,
                            lambda di, qh=qh: qh[:, di * P:(di + 1) * P],
                            DT)
                    return qT

                def emit_step1(qT, ei_range=None, qTr=None):
                    # qTr[e, l] = W_inT.T @ qT (f16), e on partitions.
                    if qTr is None:
                        qTr = actsp.tile([P, DT, LH], f16, tag="qTr", bufs=2)
                    for ei in (ei_range if ei_range is not None else range(DT)):
                        psq = ps_mm.tile([P, LH], f32, tag="mm")
                        for di in range(DT):
                            nc.tensor.matmul(
                                psq[:],
                                W_inT[:, di, ei * P:(ei + 1) * P],
                                qT[:, di, :],
                                start=(di == 0), stop=(di == DT - 1),
                            )
                        grouped_copy(nc, qTr[:, ei, :], psq[:])
                    return qTr

                def emit_half(b, h, qTr, next_bh, tail_hook=None):
                    l0 = h * LH
                    ctxT, ctx_bf = ctx_tiles[b]

                    # Prefetch the next half's query tiles first so their
                    # DMAs/casts finish before the tail needs the transposes.
                    next_qhs = (emit_query_loads(*next_bh)
                                if next_bh is not None else None)

                    # ---- step 2 + masked softmax; wT transposes lag wlag
                    # l-tiles so the softmax chain hides under the next
                    # tiles' score matmuls. w is pre-normalized by
                    # rec = 1/sum(e) on ACT before the transpose. ----
                    wT = actsp.tile([P, KT, LH], f16, tag="wT")
                    w_tiles = [None] * LJ

                    def emit_w_transpose(lj):
                        w_sb = w_tiles[lj]
                        for g in range(KT // 4):
                            tpb = ps_mm.tile([P, 4 * P], f16, tag="mm")
                            for i in range(4):
                                ki = g * 4 + i
                                nc.tensor.transpose(
                                    tpb[:, i * P:(i + 1) * P],
                                    w_sb[:, ki * P:(ki + 1) * P], ident_bf[:])
                            grouped_copy(
                                nc,
                                wT[:, g * 4:(g + 1) * 4, lj * P:(lj + 1) * P],
                                tpb[:])

                    for lj in range(LJ):
                        mi = smp.tile([P, K], i32, tag="mask", bufs=2)
                        nc.sync.dma_start(
                            mi[:], m_d[b, l0 + lj * P: l0 + (lj + 1) * P, :])
                        pss = ps_big.tile([P, K], f32, tag="big")
                        for ei in range(DT):
                            for kh in range(KH):
                                nc.tensor.matmul(
                                    pss[:, kh * 512:(kh + 1) * 512],
                                    qTr[:, ei, lj * P:(lj + 1) * P],
                                    ctxT[:, ei, kh * 512:(kh + 1) * 512],
                                    start=(ei == 0), stop=(ei == DT - 1),
                                )
                        st = smp.tile([P, 4], f32, tag="stats", bufs=2)
                        # u = (s + LARGE) * m in SBUF frees the scores PSUM
                        # right after this op.
                        u_t = smp.tile([P, K], f32, tag="u", bufs=2)
                        nc.vector.scalar_tensor_tensor(
                            u_t[:], pss[:], LARGE, mi[:],
                            op0=Alu.add, op1=Alu.mult)
                        nc.vector.tensor_reduce(
                            st[:, 0:1], u_t[:], axis=mybir.AxisListType.X,
                            op=Alu.max, negate=True)
                        e_sb = smp.tile([P, K], f16, tag="e", bufs=2)
                        nc.scalar.activation(
                            e_sb[:], u_t[:], Act.Exp,
                            bias=st[:, 0:1], accum_out=st[:, 1:2])
                        nc.vector.reciprocal(st[:, 2:3], st[:, 1:2])
                        # w = e * rec (pre-normalized so step 5 is a single
                        # PSUM accumulation).
                        w_sb = smp.tile([P, K], f16, tag="w", bufs=3)
                        nc.scalar.activation(
                            w_sb[:], e_sb[:], Act.Copy, scale=st[:, 2:3])
                        w_tiles[lj] = w_sb
                        if lj >= wlag:
                            emit_w_transpose(lj - wlag)

                    # Lagged tail. PE queue order matters (FIFO head-of-line):
                    # ready work (qT packs, step1) must come BEFORE wT(lj3),
                    # whose softmax chain is still draining. wT(lj3) slots
                    # between step1 groups so it lands just before step4.
                    if tail_hook is not None:
                        tail_hook()
                    qT_next = (emit_query_transposes(next_qhs)
                               if next_qhs is not None else None)
                    if wlag == 2:
                        emit_w_transpose(LJ - 2)
                    if qT_next is not None:
                        qTr_next = emit_step1(qT_next, ei_range=range(0, 6))
                        emit_w_transpose(LJ - 1)
                        emit_step1(qT_next, ei_range=range(6, DT), qTr=qTr_next)
                    else:
                        qTr_next = None
                        emit_w_transpose(LJ - 1)

                    # ---- step 4: mixT[d', l] = ctx_bf.T @ wT (f16) ----
                    mixT = actsp.tile([P, DT, LH], f16, tag="mixT")
                    for di in range(DT):
                        psm = ps_mm.tile([P, LH], f32, tag="mm")
                        for ki in range(KT):
                            nc.tensor.matmul(
                                psm[:],
                                ctx_bf[:, ki, di * P:(di + 1) * P],
                                wT[:, ki, :],
                                start=(ki == 0), stop=(ki == KT - 1),
                            )
                        grouped_copy(nc, mixT[:, di, :], psm[:])

                    # ---- step 5: out = tanh([mixT; qTr] @ W_outT), one PSUM
                    # accumulation per lj (w pre-normalized). ----
                    for lj in range(LJ):
                        pso = ps_big.tile([P, K], f32, tag="big")
                        for ci in range(DT):
                            lhs = mixT[:, ci, lj * P:(lj + 1) * P]
                            for dh in range(D // 512):
                                nc.tensor.matmul(
                                    pso[:, dh * 512:(dh + 1) * 512], lhs,
                                    W_outT[:, ci, dh * 512:(dh + 1) * 512],
                                    start=(ci == 0), stop=False,
                                )
                        for ci in range(DT):
                            lhs = qTr[:, ci, lj * P:(lj + 1) * P]
                            for dh in range(D // 512):
                                nc.tensor.matmul(
                                    pso[:, dh * 512:(dh + 1) * 512], lhs,
                                    W_outT[:, DT + ci,
                                           dh * 512:(dh + 1) * 512],
                                    start=False, stop=(ci == DT - 1),
                                )
                        for dh in range(D // 512):
                            o_sb = smp.tile([P, 512], f32, tag="osb", bufs=2)
                            nc.scalar.activation(
                                o_sb[:], pso[:, dh * 512:(dh + 1) * 512],
                                Act.Tanh)
                            nc.sync.dma_start(
                                out_d[b, l0 + lj * P: l0 + (lj + 1) * P,
                                      dh * 512:(dh + 1) * 512],
                                o_sb[:])
                    return qTr_next

                def emit_w_in_setup():
                    for ei in range(DT):
                        nat = natp.tile([P, D], f32, tag="nat")
                        nc.sync.dma_start(nat[:], win_d[ei * P:(ei + 1) * P, :])
                        nat16 = natp.tile([P, D], f16, tag="nat16", bufs=2)
                        nc.vector.tensor_copy(nat16[:], nat[:])
                        transpose_pack4(
                            nc, W_inT, 0, ei * P,
                            lambda di, t=nat16: t[:, di * P:(di + 1) * P],
                            DT)

                def emit_w_out_setup():
                    for di in range(DT):
                        for half in range(2):
                            nat = natp.tile([P, D], f32, tag="nat")
                            nc.sync.dma_start(
                                nat[:],
                                wout_d[di * P:(di + 1) * P,
                                       half * D:(half + 1) * D])
                            nat16 = natp.tile([P, D], f16, tag="nat16", bufs=2)
                            nc.vector.tensor_copy(nat16[:], nat[:])
                            transpose_pack4(
                                nc, W_outT, 8 * half, di * P,
                                lambda ci, t=nat16: t[:, ci * P:(ci + 1) * P],
                                DT)

                def emit_all():
                    emit_w_in_setup()
                    emit_ctx_stage(0)
                    qT = emit_query_transposes(emit_query_loads(0, 0))
                    qTr = emit_step1(qT)
                    halves = [(b, h) for b in range(nb) for h in range(NHALF)]
                    for i, (b, h) in enumerate(halves):
                        if h == 0 and b > 0:
                            emit_ctx_stage(b)
                            ctx_tiles.pop(b - 1)
                        nxt = halves[i + 1] if i + 1 < len(halves) else None
                        # W_out is only needed by step5 of the first half;
                        # emitting it in that half's tail keeps its 8 MB of
                        # DMA behind the first masks/queries in the queue.
                        hook = emit_w_out_setup if i == 0 else None
                        qTr = emit_half(b, h, qTr, nxt, tail_hook=hook)

                if reps == 1:
                    emit_all()
                else:
                    with tc.For_i(0, reps, 1):
                        emit_all()

    nc.compile()
    return nc


def _get_program(nb, L):
    key = (nb, L)
    if key not in _prog_cache:
        _prog_cache[key] = build_program(nb, L)
    return _prog_cache[key]


def kernel(query, context, mask, W_in, W_out):
    from concourse.bass_utils import run_bass_kernel_spmd

    query = np.ascontiguousarray(query, dtype=np.float32)
    context = np.ascontiguousarray(context, dtype=np.float32)
    W_in = np.ascontiguousarray(W_in, dtype=np.float32)
    W_out = np.ascontiguousarray(W_out, dtype=np.float32)
    B, L, _ = query.shape
    mask3 = np.ascontiguousarray(mask.reshape(B, L, -1), dtype=np.int32)

    nb = B // N_CORES
    nc = _get_program(nb, L)
    in_maps = []
    for c in range(N_CORES):
        b0 = c * nb
        in_maps.append({
            "query": query[b0:b0 + nb],
            "context": context[b0:b0 + nb],
            "mask": mask3[b0:b0 + nb],
            "W_in": W_in,
            "W_out": W_out,
        })
    res = run_bass_kernel_spmd(nc, in_maps, core_ids=list(range(N_CORES)))
    global last_results
    last_results = res
    out = np.concatenate([r["out"] for r in res.results], axis=0)
    return out


# revision 32
# speedup vs baseline: 1.7195x; 1.0189x over previous
"""Bass/Tile kernel for nn_Attention_9234179687166 on 8 TRN2 NeuronCores.

Reference computation per batch b (B=32, L=K=D=1024):
    q      = query @ W_in.T                    # [L, D]
    scores = q @ context.T                     # [L, K]
    w      = masked_softmax(scores, mask)      # multiplicative mask + renorm
    mix    = w @ context                       # [L, D]
    out    = tanh(concat([mix, q]) @ W_out.T)  # [L, D]

Sharding: data-parallel over batch, 4 batches per core, weights replicated.

Per-core program layout (contraction dim always on partitions):
    W_inT[d,e], W_outT[c,d] built once by PE transpose (f16 source - the
    f32->f16 cast happens right behind the weight DMAs, so the setup
    transposes run at the fast f16 weight-load rate).
    Per batch: ctxT[e,k] + ctx_bf[k,d'] (f16). Per l-half: qT[d,l],
    step1 -> qTr[e,l], step2 scores in PSUM, masked softmax (DVE+ACT),
    w = e * (1/sum(e)) pre-normalized on DVE, w transposed to wT[k,l],
    step4 -> mixT[d',l], step5 accumulates BOTH [mixT; qTr] @ W_outT parts
    into one PSUM group and applies tanh straight out of PSUM.

All transposes are 4-packed PE transposes (four 128x128 transposes into one
PSUM tile, one grouped DVE/ACT copy out). DMA-xbar transposes were measured
to anti-overlap with regular DMA traffic (1.6x worse than serial), so
everything stays on the PE where it overlaps the load stream.

Masked softmax (mask m in {0,1}, scores s):
    u = (s + 4096)*m  (masked -> 0); e = exp(u - max(u)) has masked lanes
    exp(-~4096) == 0 exactly, and w = e/sum(e) matches the reference up to
    the +1e-13 term (~1e-10 relative). w is normalized BEFORE the wT
    transpose (DVE tensor_scalar with per-partition rec), which lets step 5
    run as a single PSUM accumulation with no separate combine pass.

Software pipeline per half h (PE priority order):
    scores(h) lj=0..3 (wT transposes lag 2 tiles behind the softmax chain)
    -> qT transposes for h+1 -> step1(h+1)  [covers the softmax tail]
    -> step4(h) -> step5(h).
qTr/qT are double-buffered so step1(h+1) can run while step5(h) still
reads qTr(h).
"""

import sys

sys.path.insert(0, "/opt/trn_rl_repo")

import numpy as np

P = 128
D = 1024
TWO_D = 2048
DT = D // P      # 8 tiles over D
CT = TWO_D // P  # 16 tiles over 2D
LARGE = 4096.0
N_CORES = 8
B_FULL = 32
NB = B_FULL // N_CORES  # batches per core

_prog_cache = {}
last_results = None  # BassKernelResults of the most recent kernel() call


def build_program(nb, L, K=1024, reps=1, wlag=2):
    import concourse.mybir as mybir
    import concourse.tile as tile
    from concourse import bacc
    from concourse.masks import make_identity

    f32 = mybir.dt.float32
    f16 = mybir.dt.float16
    i32 = mybir.dt.int32
    Alu = mybir.AluOpType
    Act = mybir.ActivationFunctionType
    KT = K // P
    LH = min(512, L)      # l-half width (free dim of step1/4 matmuls)
    NHALF = L // LH
    LJ = LH // P          # 128-row l tiles per half
    KH = K // 512         # 512-wide k chunks for the scores matmul

    nc = bacc.Bacc("TRN2", target_bir_lowering=False, debug=False,
                   num_devices=N_CORES)
    q_d = nc.dram_tensor("query", [nb, L, D], f32, kind="ExternalInput")
    c_d = nc.dram_tensor("context", [nb, K, D], f32, kind="ExternalInput")
    m_d = nc.dram_tensor("mask", [nb, L, K], i32, kind="ExternalInput")
    win_d = nc.dram_tensor("W_in", [D, D], f32, kind="ExternalInput")
    wout_d = nc.dram_tensor("W_out", [D, TWO_D], f32, kind="ExternalInput")
    out_d = nc.dram_tensor("out", [nb, L, D], f32, kind="ExternalOutput")

    copy_flip = [0]

    def grouped_copy(nc, dst_ap, src_ap):
        # Alternate psum->sbuf copies between DVE and ACT to halve the
        # per-engine copy latency chain behind the PE transposes.
        if copy_flip[0] % 2 == 0:
            nc.vector.tensor_copy(dst_ap, src_ap)
        else:
            nc.scalar.activation(dst_ap, src_ap, mybir.ActivationFunctionType.Copy)
        copy_flip[0] += 1

    with tile.TileContext(nc) as tc:
        with (
            tc.tile_pool(name="const", bufs=1) as constp,
            tc.tile_pool(name="wres", bufs=1) as wres,
            tc.tile_pool(name="ps_big", bufs=2, space="PSUM") as ps_big,
            tc.tile_pool(name="ps_mm", bufs=4, space="PSUM") as ps_mm,
        ):
            ident = constp.tile([P, P], f32)
            make_identity(nc, ident)
            ident_bf = constp.tile([P, P], f16)
            nc.vector.tensor_copy(ident_bf[:], ident[:])

            W_inT = wres.tile([P, DT, D], f16)        # [d_in, d_out, e]
            W_outT = wres.tile([P, CT, D], f16)      # [c_in, c_out, d]

            def transpose_pack4(nc, dst_tile, dst_t0, dst_col0, src_ap_fn, n):
                """n f16 PE transposes (groups of up to 4) of 128x128 slices.
                src_ap_fn(i) gives the i-th source slice; results land in
                dst_tile[:, dst_t0+i, dst_col0:dst_col0+128]."""
                g = 0
                while g < n:
                    gn = min(4, n - g)
                    tp = ps_mm.tile([P, 4 * P], f16, tag="mm")
                    for i in range(gn):
                        nc.tensor.transpose(
                            tp[:, i * P:(i + 1) * P], src_ap_fn(g + i),
                            ident_bf[:])
                    grouped_copy(
                        nc,
                        dst_tile[:, dst_t0 + g:dst_t0 + g + gn,
                                 dst_col0:dst_col0 + P],
                        tp[:, :gn * P],
                    )
                    g += gn

            with (
                tc.tile_pool(name="ctx", bufs=1) as ctxp,
                tc.tile_pool(name="acts", bufs=1) as actsp,
                tc.tile_pool(name="rot", bufs=4) as natp,
                tc.tile_pool(name="sm", bufs=2) as smp,
            ):
                ctx_tiles = {}

                def emit_ctx_stage(b):
                    # context: cast to f16 (DVE - it is idle in the
                    # step5/step1 window where these become ready; ACT is
                    # busy with tanh there), PE-transpose to ctxT.
                    ctxT = ctxp.tile([P, DT, K], f16, tag="ctxT")      # [e,., k]
                    ctx_bf = ctxp.tile([P, KT, D], f16, tag="ctxbf")  # [k,., d']
                    for ki in range(KT):
                        nat = natp.tile([P, D], f32, tag="nat")
                        nc.sync.dma_start(nat[:], c_d[b, ki * P:(ki + 1) * P, :])
                        nc.vector.tensor_copy(ctx_bf[:, ki, :], nat[:])
                        transpose_pack4(
                            nc, ctxT, 0, ki * P,
                            lambda ei, ki=ki: ctx_bf[:, ki, ei * P:(ei + 1) * P],
                            DT)
                    ctx_tiles[b] = (ctxT, ctx_bf)

                def emit_query_loads(b, h):
                    # Cast each chunk to f16 (DVE) right behind its DMA so
                    # the transposes run single-pass and the nat rotor frees
                    # early.
                    l0 = h * LH
                    qhs = []
                    for lj in range(LJ):
                        nat = natp.tile([P, D], f32, tag="nat")
                        nc.sync.dma_start(
                            nat[:], q_d[b, l0 + lj * P: l0 + (lj + 1) * P, :])
                        qh = smp.tile([P, D], f16, tag="qh", bufs=4)
                        nc.vector.tensor_copy(qh[:], nat[:])
                        qhs.append(qh)
                    return qhs

                def emit_query_transposes(qhs):
                    qT = actsp.tile([P, DT, LH], f16, tag="qT", bufs=2)
                    for lj, qh in enumerate(qhs):
                        transpose_pack4(
                            nc, qT, 0, lj * P,
                            lambda di, qh=qh: qh[:, di * P:(di + 1) * P],
                            DT)
                    return qT

                def emit_step1(qT, ei_range=None, qTr=None):
                    # qTr[e, l] = W_inT.T @ qT (f16), e on partitions.
                    if qTr is None:
                        qTr = actsp.tile([P, DT, LH], f16, tag="qTr", bufs=2)
                    for ei in (ei_range if ei_range is not None else range(DT)):
                        psq = ps_mm.tile([P, LH], f32, tag="mm")
                        for di in range(DT):
                            nc.tensor.matmul(
                                psq[:],
                                W_inT[:, di, ei * P:(ei + 1) * P],
                                qT[:, di, :],
                                start=(di == 0), stop=(di == DT - 1),
                            )
                        grouped_copy(nc, qTr[:, ei, :], psq[:])
                    return qTr

                def emit_half(b, h, qTr, next_bh, tail_hook=None):
                    l0 = h * LH
                    ctxT, ctx_bf = ctx_tiles[b]

                    # Prefetch the next half's query tiles first so their
                    # DMAs/casts finish before the tail needs the transposes.
                    next_qhs = (emit_query_loads(*next_bh)
                                if next_bh is not None else None)

                    # ---- step 2 + masked softmax; wT transposes lag wlag
                    # l-tiles so the softmax chain hides under the next
                    # tiles' score matmuls. w is pre-normalized by
                    # rec = 1/sum(e) on ACT before the transpose. ----
                    wT = actsp.tile([P, KT, LH], f16, tag="wT")
                    w_tiles = [None] * LJ

                    def emit_w_transpose(lj):
                        w_sb = w_tiles[lj]
                        for g in range(KT // 4):
                            tpb = ps_mm.tile([P, 4 * P], f16, tag="mm")
                            for i in range(4):
                                ki = g * 4 + i
                                nc.tensor.transpose(
                                    tpb[:, i * P:(i + 1) * P],
                                    w_sb[:, ki * P:(ki + 1) * P], ident_bf[:])
                            grouped_copy(
                                nc,
                                wT[:, g * 4:(g + 1) * 4, lj * P:(lj + 1) * P],
                                tpb[:])

                    for lj in range(LJ):
                        mi = smp.tile([P, K], i32, tag="mask", bufs=2)
                        nc.sync.dma_start(
                            mi[:], m_d[b, l0 + lj * P: l0 + (lj + 1) * P, :])
                        pss = ps_big.tile([P, K], f32, tag="big")
                        for ei in range(DT):
                            for kh in range(KH):
                                nc.tensor.matmul(
                                    pss[:, kh * 512:(kh + 1) * 512],
                                    qTr[:, ei, lj * P:(lj + 1) * P],
                                    ctxT[:, ei, kh * 512:(kh + 1) * 512],
                                    start=(ei == 0), stop=(ei == DT - 1),
                                )
                        st = smp.tile([P, 4], f32, tag="stats", bufs=2)
                        # u = (s + LARGE) * m in SBUF frees the scores PSUM
                        # right after this op.
                        u_t = smp.tile([P, K], f32, tag="u", bufs=2)
                        nc.vector.scalar_tensor_tensor(
                            u_t[:], pss[:], LARGE, mi[:],
                            op0=Alu.add, op1=Alu.mult)
                        nc.vector.tensor_reduce(
                            st[:, 0:1], u_t[:], axis=mybir.AxisListType.X,
                            op=Alu.max, negate=True)
                        e_sb = smp.tile([P, K], f16, tag="e", bufs=2)
                        nc.scalar.activation(
                            e_sb[:], u_t[:], Act.Exp,
                            bias=st[:, 0:1], accum_out=st[:, 1:2])
                        nc.vector.reciprocal(st[:, 2:3], st[:, 1:2])
                        # w = e * rec (pre-normalized so step 5 is a single
                        # PSUM accumulation), on DVE: ACT is the tighter
                        # engine during the scores loop.
                        w_sb = smp.tile([P, K], f16, tag="w", bufs=3)
                        nc.vector.tensor_scalar(
                            w_sb[:], e_sb[:], st[:, 2:3], None, op0=Alu.mult)
                        w_tiles[lj] = w_sb
                        if lj >= wlag:
                            emit_w_transpose(lj - wlag)

                    # Lagged tail. PE queue order matters (FIFO head-of-line):
                    # ready work (qT packs, step1) must come BEFORE wT(lj3),
                    # whose softmax chain is still draining. wT(lj3) slots
                    # between step1 groups so it lands just before step4.
                    if tail_hook is not None:
                        tail_hook()
                    qT_next = (emit_query_transposes(next_qhs)
                               if next_qhs is not None else None)
                    if wlag == 2:
                        emit_w_transpose(LJ - 2)
                    if qT_next is not None:
                        qTr_next = emit_step1(qT_next, ei_range=range(0, 6))
                        emit_w_transpose(LJ - 1)
                        emit_step1(qT_next, ei_range=range(6, DT), qTr=qTr_next)
                    else:
                        qTr_next = None
                        emit_w_transpose(LJ - 1)

                    # ---- step 4: mixT[d', l] = ctx_bf.T @ wT (f16) ----
                    mixT = actsp.tile([P, DT, LH], f16, tag="mixT")
                    for di in range(DT):
                        psm = ps_mm.tile([P, LH], f32, tag="mm")
                        for ki in range(KT):
                            nc.tensor.matmul(
                                psm[:],
                                ctx_bf[:, ki, di * P:(di + 1) * P],
                                wT[:, ki, :],
                                start=(ki == 0), stop=(ki == KT - 1),
                            )
                        grouped_copy(nc, mixT[:, di, :], psm[:])

                    # ---- step 5: out = tanh([mixT; qTr] @ W_outT), one PSUM
                    # accumulation per lj (w pre-normalized). ----
                    for lj in range(LJ):
                        pso = ps_big.tile([P, K], f32, tag="big")
                        for ci in range(DT):
                            lhs = mixT[:, ci, lj * P:(lj + 1) * P]
                            for dh in range(D // 512):
                                nc.tensor.matmul(
                                    pso[:, dh * 512:(dh + 1) * 512], lhs,
                                    W_outT[:, ci, dh * 512:(dh + 1) * 512],
                                    start=(ci == 0), stop=False,
                                )
                        for ci in range(DT):
                            lhs = qTr[:, ci, lj * P:(lj + 1) * P]
                            for dh in range(D // 512):
                                nc.tensor.matmul(
                                    pso[:, dh * 512:(dh + 1) * 512], lhs,
                                    W_outT[:, DT + ci,
                                           dh * 512:(dh + 1) * 512],
                                    start=False, stop=(ci == DT - 1),
                                )
                        for dh in range(D // 512):
                            o_sb = smp.tile([P, 512], f32, tag="osb", bufs=2)
                            nc.scalar.activation(
                                o_sb[:], pso[:, dh * 512:(dh + 1) * 512],
                                Act.Tanh)
                            nc.sync.dma_start(
                                out_d[b, l0 + lj * P: l0 + (lj + 1) * P,
                                      dh * 512:(dh + 1) * 512],
                                o_sb[:])
                    return qTr_next

                def emit_w_in_setup():
                    for ei in range(DT):
                        nat = natp.tile([P, D], f32, tag="nat")
                        nc.sync.dma_start(nat[:], win_d[ei * P:(ei + 1) * P, :])
                        nat16 = natp.tile([P, D], f16, tag="nat16", bufs=2)
                        nc.vector.tensor_copy(nat16[:], nat[:])
                        transpose_pack4(
                            nc, W_inT, 0, ei * P,
                            lambda di, t=nat16: t[:, di * P:(di + 1) * P],
                            DT)

                def emit_w_out_setup():
                    for di in range(DT):
                        for half in range(2):
                            nat = natp.tile([P, D], f32, tag="nat")
                            nc.sync.dma_start(
                                nat[:],
                                wout_d[di * P:(di + 1) * P,
                                       half * D:(half + 1) * D])
                            nat16 = natp.tile([P, D], f16, tag="nat16", bufs=2)
                            nc.vector.tensor_copy(nat16[:], nat[:])
                            transpose_pack4(
                                nc, W_outT, 8 * half, di * P,
                                lambda ci, t=nat16: t[:, ci * P:(ci + 1) * P],
                                DT)

                def emit_all():
                    emit_w_in_setup()
                    emit_ctx_stage(0)
                    qT = emit_query_transposes(emit_query_loads(0, 0))
                    qTr = emit_step1(qT)
                    halves = [(b, h) for b in range(nb) for h in range(NHALF)]
                    for i, (b, h) in enumerate(halves):
                        if h == 0 and b > 0:
                            emit_ctx_stage(b)
                            ctx_tiles.pop(b - 1)
                        nxt = halves[i + 1] if i + 1 < len(halves) else None
                        # W_out is only needed by step5 of the first half;
                        # emitting it in that half's tail keeps its 8 MB of
                        # DMA behind the first masks/queries in the queue.
                        hook = emit_w_out_setup if i == 0 else None
                        qTr = emit_half(b, h, qTr, nxt, tail_hook=hook)

                if reps == 1:
                    emit_all()
                else:
                    with tc.For_i(0, reps, 1):
                        emit_all()

    nc.compile()
    return nc


def _get_program(nb, L):
    key = (nb, L)
    if key not in _prog_cache:
        _prog_cache[key] = build_program(nb, L)
    return _prog_cache[key]


def kernel(query, context, mask, W_in, W_out):
    from concourse.bass_utils import run_bass_kernel_spmd

    query = np.ascontiguousarray(query, dtype=np.float32)
    context = np.ascontiguousarray(context, dtype=np.float32)
    W_in = np.ascontiguousarray(W_in, dtype=np.float32)
    W_out = np.ascontiguousarray(W_out, dtype=np.float32)
    B, L, _ = query.shape
    mask3 = np.ascontiguousarray(mask.reshape(B, L, -1), dtype=np.int32)

    nb = B // N_CORES
    nc = _get_program(nb, L)
    in_maps = []
    for c in range(N_CORES):
        b0 = c * nb
        in_maps.append({
            "query": query[b0:b0 + nb],
            "context": context[b0:b0 + nb],
            "mask": mask3[b0:b0 + nb],
            "W_in": W_in,
            "W_out": W_out,
        })
    res = run_bass_kernel_spmd(nc, in_maps, core_ids=list(range(N_CORES)))
    global last_results
    last_results = res
    out = np.concatenate([r["out"] for r in res.results], axis=0)
    return out
